# revision 33
# baseline (speedup 1.0000x reference)
"""Trainium2 Bass kernel for causal MultiHeadAttention + residual + LayerNorm.

Problem shapes (hardcoded):
  B=4, S=2048, D_MODEL=1024, H=8 heads, d_k=128.
  out = LayerNorm(queries + MHA(LN-free)(queries, keys, values))

Sharding (8 cores):
  Launch 1 (attention): core c <-> (batch b = c//2, head group g = c%2 -> heads
  4g..4g+3).  Q/K/V weights column-sharded by head group; X^T passed
  pre-transposed in bf16.  Each core computes its 4 heads' attention output
  O^T [4,128,2048] f32.
  Launch 2 (layernorm): row-sharded, 1024 rows of the flattened [8192,1024]
  residual per core.
"""

import sys

import numpy as np

for _p in ("/opt/trn_rl_repo", "/opt/pypackages"):
    if _p not in sys.path:
        sys.path.append(_p)

import ml_dtypes  # noqa: E402

import concourse.bass as bass  # noqa: E402
import concourse.mybir as mybir  # noqa: E402
import concourse.tile as tile_mod  # noqa: E402
from concourse.tile import TileContext  # noqa: E402
from concourse.bass_utils import run_bass_kernel_spmd  # noqa: E402
from concourse.masks import make_lower_triangular  # noqa: E402

B = 4
S = 2048
D = 1024
H = 8
DK = 128
HG = 4  # heads per core
NCORES = 8
SCALE = 1.0 / np.sqrt(np.float32(DK))
NEG_INF = -1e9
EPS = 1e-6

BF16 = mybir.dt.bfloat16
F32 = mybir.dt.float32
NPBF16 = ml_dtypes.bfloat16

_PATCHED = False


def _bcast_rows(ap):
    """Broadcast a 1-D dram AP across 128 partitions (step-0 partition dim)."""
    return bass.AP(tensor=ap.tensor, offset=ap.offset, ap=[[0, 128]] + list(ap.ap))


def _patch_tile_drain():
    # retained for API compatibility; wait splitting now happens in
    # _split_excess_waits after scheduling.
    return


def _split_excess_waits(nc):
    """Workaround for this walrus build: engine (TPB) instructions accept at
    most one sync-wait command (EventSemaphore: two), but Tile attaches one
    wait per dependency.  Move excess waits onto same-engine NOPs inserted
    immediately before the over-limit instruction — the engine executes
    in-order, so stalling at the NOP(s) first is semantically identical.
    DMA/collective instructions are exempt (queue descriptors support
    multiple waits)."""
    n_new = 0
    for f in nc.m.functions:
        for bb in f.blocks:
            il = bb.instructions
            out = []
            changed = False
            for ins in il:
                si = ins.sync_info
                tname = type(ins).__name__
                if si is not None:
                    cap = 2 if tname == "InstEventSemaphore" else 1
                    waits = list(si.on_wait)
                    if len(waits) > cap:
                        for w in waits[cap:]:
                            nop = mybir.InstNoOp(
                                name=f"I-wsplit-{n_new}",
                                sync_info=mybir.SyncInfo(
                                    on_wait=[w], on_update=[]
                                ),
                                bass_nofuse=True,
                                engine=ins.engine,
                            )
                            n_new += 1
                            out.append(nop)
                        si.on_wait = waits[:cap]
                        changed = True
                out.append(ins)
            if changed:
                il[:] = out
    return n_new


def _build_attention():
    """Per-core attention program: 4 heads of one batch.

    Structure: V projection, then K^T projection (all 4 heads), then per
    head: Q^T projection immediately followed by that head's attention —
    so ScalarE exp work overlaps the next head's projection matmuls.

    Outputs:
      o_t : [HG, DK, S] bf16 -- per-head UNNORMALIZED attention output O^T
      rs  : [HG, S]     f32  -- per-head softmax row sums (denominators)
    """
    nc = bass.Bass()

    NSC = S // 512
    KCC = D // 128
    # activations pre-chunked on host: [sc, 128, kc, 512]
    xq_t = nc.dram_tensor("xq_t", [NSC, 128, KCC, 512], BF16, kind="ExternalInput")
    xk_t = nc.dram_tensor("xk_t", [NSC, 128, KCC, 512], BF16, kind="ExternalInput")
    xv_t = nc.dram_tensor("xv_t", [NSC, 128, KCC, 512], BF16, kind="ExternalInput")
    # weights pre-permuted on host: [128, kc, 4*DK]
    wq = nc.dram_tensor("wq", [128, KCC, HG * DK], BF16, kind="ExternalInput")
    wk = nc.dram_tensor("wk", [128, KCC, HG * DK], BF16, kind="ExternalInput")
    wv = nc.dram_tensor("wv", [128, KCC, HG * DK], BF16, kind="ExternalInput")
    # biases pre-shaped on host: bq/bk [128, HG]; bv broadcast [128, HG*DK]
    bq = nc.dram_tensor("bq", [128, HG], F32, kind="ExternalInput")
    bk = nc.dram_tensor("bk", [128, HG], F32, kind="ExternalInput")
    bv = nc.dram_tensor("bv", [128, HG * DK], F32, kind="ExternalInput")
    o_t = nc.dram_tensor("o_t", [HG, DK, S], BF16, kind="ExternalOutput")
    rs = nc.dram_tensor("rs", [HG, S], F32, kind="ExternalOutput")

    KC = D // 128          # 8 contraction chunks
    NS = S // 512          # 4 s-chunks of 512
    NJ = S // 128          # 16 key chunks
    HW = HG * DK           # 512

    with TileContext(nc) as tc:
        from contextlib import ExitStack

        with ExitStack() as ctx:
            consts = ctx.enter_context(tc.tile_pool(name="consts", bufs=1))
            proj_out = ctx.enter_context(tc.tile_pool(name="proj_out", bufs=1))
            wpool = ctx.enter_context(tc.tile_pool(name="w", bufs=2))
            xspool = ctx.enter_context(tc.tile_pool(name="xs", bufs=2))
            xqpool = ctx.enter_context(tc.tile_pool(name="xq", bufs=1))
            ptpool = ctx.enter_context(tc.tile_pool(name="pt", bufs=1))
            osbpool = ctx.enter_context(tc.tile_pool(name="osb", bufs=4))
            rsspool = ctx.enter_context(tc.tile_pool(name="rss", bufs=1))
            stpool = ctx.enter_context(
                tc.tile_pool(name="st", bufs=2, space="PSUM")
            )
            rspool = ctx.enter_context(
                tc.tile_pool(name="rsp", bufs=1, space="PSUM")
            )
            otpool = ctx.enter_context(
                tc.tile_pool(name="ot", bufs=2, space="PSUM")
            )

            # --- constants ---
            tril = consts.tile([128, 128], F32)  # additive: -1e9 where k > q
            make_lower_triangular(nc, tril, val=NEG_INF, diag=False)
            ones_b = consts.tile([128, 1], BF16)
            nc.vector.memset(ones_b, 1.0)
            bq_sb = consts.tile([128, HG], F32)
            bk_sb = consts.tile([128, HG], F32)
            nc.sync.dma_start(out=bq_sb, in_=bq[:])
            nc.sync.dma_start(out=bk_sb, in_=bk[:])
            bv_sb = consts.tile([128, HW], F32)
            nc.sync.dma_start(out=bv_sb, in_=bv[:])

            # --- projection outputs ---
            qt_sb = [proj_out.tile([128, S], BF16, tag=f"qt{h}", name=f"qt{h}") for h in range(HG)]
            kt_sb = [proj_out.tile([128, S], BF16, tag=f"kt{h}", name=f"kt{h}") for h in range(HG)]
            v_sb = proj_out.tile([128, NJ, HW], BF16, tag="v", name="v")

            def load_w(w_d, name):
                w_t = wpool.tile([128, KC, HW], BF16, tag="w", name=name)
                nc.sync.dma_start(out=w_t, in_=w_d[:])
                return w_t

            def load_x_chunk(x_d, sc, name):
                # two sub-DMAs so the transfer spreads over two HW queues
                xs = xspool.tile([128, KC, 512], BF16, tag="xs", name=name)
                nc.sync.dma_start(out=xs[:, :KC // 2], in_=x_d[sc, :, :KC // 2])
                nc.sync.dma_start(out=xs[:, KC // 2:], in_=x_d[sc, :, KC // 2:])
                return xs

            # --- V projection: out[s, d_head] = X^T.T @ W ---
            w_t = load_w(wv, "wv_t")
            vproj_last = None
            for sc in range(NS):
                xs = load_x_chunk(xv_t, sc, f"xv{sc}")
                for sbl in range(4):
                    sb = 4 * sc + sbl
                    ps = stpool.tile([128, 512], F32, tag="st", name="psv")
                    for kc in range(KC):
                        mm = nc.tensor.matmul(
                            ps,
                            lhsT=xs[:, kc, sbl * 128:(sbl + 1) * 128],
                            rhs=w_t[:, kc, :],
                            start=(kc == 0),
                            stop=(kc == KC - 1),
                        )
                        vproj_last = mm
                    nc.vector.tensor_add(out=v_sb[:, sb, :], in0=ps, in1=bv_sb)

            # --- K^T projection (all heads) ---
            w_t = load_w(wk, "wk_t")
            for sc in range(NS):
                xs = load_x_chunk(xk_t, sc, f"xk{sc}")
                for h in range(HG):
                    ps = stpool.tile([128, 512], F32, tag="st", name="psk")
                    for kc in range(KC):
                        nc.tensor.matmul(
                            ps,
                            lhsT=w_t[:, kc, h * DK:(h + 1) * DK],
                            rhs=xs[:, kc, :],
                            start=(kc == 0),
                            stop=(kc == KC - 1),
                        )
                    nc.scalar.activation(
                        out=kt_sb[h][:, sc * 512:(sc + 1) * 512],
                        in_=ps,
                        func=mybir.ActivationFunctionType.Identity,
                        bias=bk_sb[:, h:h + 1],
                    )

            # --- per head: Q^T projection + attention ---
            # (loads emitted after K-proj so startup DMA bandwidth goes to
            # the V/K activations first)
            wq_t = load_w(wq, "wq_t")
            xq_tt = xqpool.tile([128, NS, KC, 512], BF16, tag="xq", name="xq_tt")
            for sc in range(NS):
                nc.sync.dma_start(
                    out=xq_tt[:, sc, :KC // 2], in_=xq_t[sc, :, :KC // 2]
                )
                nc.sync.dma_start(
                    out=xq_tt[:, sc, KC // 2:], in_=xq_t[sc, :, KC // 2:]
                )
            for h in range(HG):
                for sc in range(NS):
                    ps = stpool.tile([128, 512], F32, tag="st", name="psq")
                    for kc in range(KC):
                        nc.tensor.matmul(
                            ps,
                            lhsT=wq_t[:, kc, h * DK:(h + 1) * DK],
                            rhs=xq_tt[:, sc, kc, :],
                            start=(kc == 0),
                            stop=(kc == KC - 1),
                        )
                    nc.scalar.activation(
                        out=qt_sb[h][:, sc * 512:(sc + 1) * 512],
                        in_=ps,
                        func=mybir.ActivationFunctionType.Identity,
                        bias=bq_sb[:, h:h + 1],
                    )

                # Phase A: S^T chunks -> exp -> P^T[j]; row sums on PE
                # (all-ones M=1 matmuls accumulating into [1,512] PSUM).
                pts = []
                bases = []
                rsp_pk = rspool.tile([128, 512], F32, tag="rsp", name=f"rsp{h}")
                def emit_rsums(j):
                    r0 = j // 4
                    jq = j * 128
                    base = bases[j]
                    for r in range(r0, NS):
                        qlo = max(r * 512, jq)
                        a = qlo - r * 512
                        nc.tensor.matmul(
                            rsp_pk[32 * r:32 * r + 1, a:512],
                            lhsT=ones_b,
                            rhs=pts[j][:, qlo - base:(r + 1) * 512 - base],
                            start=(j == 0),
                            stop=(j == 4 * r + 3),
                            tile_position=(0, 32 * r),
                        )

                for j in range(NJ):
                    r0 = j // 4
                    jq = j * 128
                    base = r0 * 512
                    pt = ptpool.tile([128, S - base], BF16, tag=f"pt{j}",
                                     name=f"pt{h}_{j}")
                    pts.append(pt)
                    bases.append(base)
                    for hl in range(r0 // 2, 2):
                        qlo = max(hl * 1024, jq)
                        a = qlo - hl * 1024
                        st = stpool.tile([128, 1024], F32, tag="st", name="st")
                        for r in range(max(2 * hl, r0), 2 * hl + 2):
                            rqlo = max(r * 512, jq)
                            ra = rqlo - hl * 1024
                            nc.tensor.matmul(
                                st[:, ra:(r + 1) * 512 - hl * 1024],
                                lhsT=kt_sb[h][:, jq:jq + 128],
                                rhs=qt_sb[h][:, rqlo:(r + 1) * 512],
                                start=True,
                                stop=True,
                            )
                        if qlo == jq:
                            nc.vector.tensor_add(
                                out=st[:, a:a + 128],
                                in0=st[:, a:a + 128],
                                in1=tril,
                            )
                        nc.scalar.activation(
                            out=pt[:, qlo - base:(hl + 1) * 1024 - base],
                            in_=st[:, a:1024],
                            func=mybir.ActivationFunctionType.Exp,
                            scale=float(SCALE),
                        )
                    if j > 0:
                        emit_rsums(j - 1)
                emit_rsums(NJ - 1)
                # export row sums
                rs_sb = rsspool.tile([1, S], F32, tag="rss", name=f"rs_sb{h}")
                for r in range(NS):
                    nc.scalar.copy(
                        out=rs_sb[:, r * 512:(r + 1) * 512],
                        in_=rsp_pk[32 * r:32 * r + 1, :],
                    )
                nc.sync.dma_start(out=rs[h:h + 1, :], in_=rs_sb[0:1, :])
                # Phase B: O^T[r] = sum_j V_j^T P^T[j], two ranges per pass.
                for half in range(2):
                    ot_pss = [
                        otpool.tile([128, 512], F32, tag="ot",
                                    name=f"ot{h}_{half}_{k}")
                        for k in range(2)
                    ]
                    rlo = 2 * half
                    for j in range(4 * (rlo + 1) + 4):
                        for k in range(2):
                            r = rlo + k
                            if j >= 4 * r + 4:
                                continue
                            qlo = max(r * 512, j * 128)
                            a = qlo - r * 512
                            nc.tensor.matmul(
                                ot_pss[k][:, a:512],
                                lhsT=v_sb[:, j, h * DK:(h + 1) * DK],
                                rhs=pts[j][:, qlo - bases[j]:(r + 1) * 512 - bases[j]],
                                start=(j == 0),
                                stop=(j == 4 * r + 3),
                            )
                    for k in range(2):
                        r = rlo + k
                        o_sb = osbpool.tile([128, 512], BF16, tag="osb",
                                            name=f"o_sb{h}_{r}")
                        nc.vector.tensor_copy(out=o_sb, in_=ot_pss[k])
                        nc.sync.dma_start(
                            out=o_t[h, :, r * 512:(r + 1) * 512], in_=o_sb
                        )
    _split_excess_waits(nc)
    return nc


def _build_layernorm(affine=True):
    """Per-core: residual add + LayerNorm over 1024 rows of [8192, 1024].

    affine=False omits the gamma/beta application (valid when gamma==1,
    beta==0, which is what this problem's setup_inputs produces)."""
    nc = bass.Bass()
    RPC = (B * S) // NCORES  # 1024 rows per core

    attn = nc.dram_tensor("attn", [RPC, D], BF16, kind="ExternalInput")
    rinv = nc.dram_tensor("rinv", [RPC, H], F32, kind="ExternalInput")
    resid = nc.dram_tensor("resid", [RPC, D], F32, kind="ExternalInput")
    gamma = nc.dram_tensor("gamma", [D], F32, kind="ExternalInput")
    beta = nc.dram_tensor("beta", [D], F32, kind="ExternalInput")
    out = nc.dram_tensor("out", [RPC, D], F32, kind="ExternalOutput")

    with TileContext(nc) as tc:
        with (
            tc.tile_pool(name="consts", bufs=1) as consts,
            tc.tile_pool(name="work", bufs=3) as work,
            tc.tile_pool(name="stat", bufs=4) as statp,
        ):
            if affine:
                gamma_sb = consts.tile([128, D], F32)
                beta_sb = consts.tile([128, D], F32)
                nc.gpsimd.dma_start(out=gamma_sb, in_=_bcast_rows(gamma[:]))
                nc.gpsimd.dma_start(out=beta_sb, in_=_bcast_rows(beta[:]))
            eps_sb = consts.tile([128, 1], F32)
            nc.vector.memset(eps_sb, EPS)

            nsub = D // 512  # bn_stats free-dim limit
            NT = RPC // 128
            for t in range(NT):
                xb = work.tile([128, D], BF16, tag="xb", name="xb")
                x = work.tile([128, D], F32, tag="x", name="x")
                rtile = work.tile([128, D], F32, tag="r", name="rtile")
                ri = work.tile([128, H], F32, tag="ri", name="ri")
                nc.sync.dma_start(out=xb, in_=attn[t * 128:(t + 1) * 128, :])
                nc.sync.dma_start(out=rtile, in_=resid[t * 128:(t + 1) * 128, :])
                nc.sync.dma_start(out=ri, in_=rinv[t * 128:(t + 1) * 128, :])
                # softmax normalization folded in: per-head column blocks (ACT)
                for hb in range(H):
                    nc.scalar.activation(
                        out=x[:, hb * DK:(hb + 1) * DK],
                        in_=xb[:, hb * DK:(hb + 1) * DK],
                        func=mybir.ActivationFunctionType.Copy,
                        scale=ri[:, hb:hb + 1],
                    )
                nc.vector.tensor_add(out=x, in0=x, in1=rtile)

                stats = statp.tile([128, nsub, 6], F32, tag="stats", name="stats")
                for sgi in range(nsub):
                    nc.vector.bn_stats(
                        out=stats[:, sgi, :], in_=x[:, sgi * 512:(sgi + 1) * 512]
                    )
                mv = statp.tile([128, 2], F32, tag="mv", name="mv")
                nc.vector.bn_aggr(out=mv, in_=stats)
                rstd = statp.tile([128, 1], F32, tag="rstd", name="rstd")
                nc.scalar.activation(
                    out=rstd,
                    in_=mv[:, 1:2],
                    func=mybir.ActivationFunctionType.Sqrt,
                    bias=eps_sb,
                    scale=1.0,
                )
                nc.vector.reciprocal(out=rstd, in_=rstd)
                nc.vector.tensor_scalar(
                    out=x,
                    in0=x,
                    scalar1=mv[:, 0:1],
                    scalar2=rstd,
                    op0=mybir.AluOpType.subtract,
                    op1=mybir.AluOpType.mult,
                )
                if affine:
                    nc.vector.tensor_mul(out=x, in0=x, in1=gamma_sb)
                    nc.vector.tensor_add(out=x, in0=x, in1=beta_sb)
                nc.sync.dma_start(out=out[t * 128:(t + 1) * 128, :], in_=x)
    _split_excess_waits(nc)
    return nc


_CACHE = {}


def _patch_ldw_opt():
    # hide LDWEIGHTS behind matmuls: walrus default here disables the
    # LDW scheduling optimization; flip the flag at the compile boundary.
    import concourse.bass_utils as bu

    if getattr(bu, "_ldw_patched", False):
        return
    orig = bu.run_command

    def run_command_ldw(argv, **kw):
        argv = [
            a
            if isinstance(a, str) else a
            for a in argv
        ]
        return orig(argv, **kw)

    bu.run_command = run_command_ldw
    bu._ldw_patched = True


def _get_programs(affine=True):
    if "attn" not in _CACHE:
        _patch_ldw_opt()
        _CACHE["attn"] = _build_attention()
    key = ("ln", affine)
    if key not in _CACHE:
        _CACHE[key] = _build_layernorm(affine=affine)
    return _CACHE["attn"], _CACHE[key]


def _run(inputs, trace=False):
    """Returns (output, attn_results, ln_results)."""
    gamma_np = np.asarray(inputs["gamma"], dtype=np.float32)
    beta_np = np.asarray(inputs["beta"], dtype=np.float32)
    affine = not (np.all(gamma_np == 1.0) and np.all(beta_np == 0.0))
    nc_attn, nc_ln = _get_programs(affine=affine)

    q = np.ascontiguousarray(np.asarray(inputs["queries"], dtype=np.float32))
    k = np.ascontiguousarray(np.asarray(inputs["keys"], dtype=np.float32))
    v = np.ascontiguousarray(np.asarray(inputs["values"], dtype=np.float32))
    Wq = np.asarray(inputs["Wq"], dtype=np.float32)
    Wk = np.asarray(inputs["Wk"], dtype=np.float32)
    Wv = np.asarray(inputs["Wv"], dtype=np.float32)
    bq = np.asarray(inputs["bq"], dtype=np.float32)
    bk = np.asarray(inputs["bk"], dtype=np.float32)
    bv = np.asarray(inputs["bv"], dtype=np.float32)
    gamma = np.asarray(inputs["gamma"], dtype=np.float32)
    beta = np.asarray(inputs["beta"], dtype=np.float32)

    # host-side shard prep: bf16 casts + kernel-native layouts
    KC, NSC = D // 128, S // 512

    def prep_x(xb):
        # [S, D] -> X^T chunks [sc, 128, kc, 512]
        xT = xb.T.astype(NPBF16)  # [D, S]
        return np.ascontiguousarray(
            xT.reshape(KC, 128, NSC, 512).transpose(2, 1, 0, 3)
        )

    def prep_w(Wm, g):
        ws = Wm[:, g * 512:(g + 1) * 512].astype(NPBF16)  # [D, 512]
        return np.ascontiguousarray(ws.reshape(KC, 128, 512).transpose(1, 0, 2))

    xt = {}
    for b in range(B):
        xt[("q", b)] = prep_x(q[b])
        xt[("k", b)] = prep_x(k[b])
        xt[("v", b)] = prep_x(v[b])
    wslices = {}
    bslices = {}
    for g in range(2):
        cols = slice(g * 512, (g + 1) * 512)
        wslices[("q", g)] = prep_w(Wq, g)
        wslices[("k", g)] = prep_w(Wk, g)
        wslices[("v", g)] = prep_w(Wv, g)
        bslices[("q", g)] = np.ascontiguousarray(bq[cols].reshape(HG, 128).T)
        bslices[("k", g)] = np.ascontiguousarray(bk[cols].reshape(HG, 128).T)
        bslices[("v", g)] = np.ascontiguousarray(
            np.broadcast_to(bv[cols], (128, 512))
        )

    in_maps = []
    for c in range(NCORES):
        b, g = c // 2, c % 2
        in_maps.append({
            "xq_t": xt[("q", b)],
            "xk_t": xt[("k", b)],
            "xv_t": xt[("v", b)],
            "wq": wslices[("q", g)],
            "wk": wslices[("k", g)],
            "wv": wslices[("v", g)],
            "bq": bslices[("q", g)],
            "bk": bslices[("k", g)],
            "bv": bslices[("v", g)],
        })

    res1 = run_bass_kernel_spmd(
        nc_attn, in_maps, core_ids=list(range(NCORES)), trace=trace
    )

    # assemble full attention output [B, S, D] and per-(b,head) rsums
    attn_full = np.empty((B, S, D), dtype=NPBF16)
    rinv_full = np.empty((B, S, H), dtype=np.float32)
    for c in range(NCORES):
        b, g = c // 2, c % 2
        ot = res1.results[c]["o_t"]  # [HG, DK, S]
        rs = res1.results[c]["rs"]  # [HG, S]
        for i in range(HG):
            attn_full[b, :, (g * HG + i) * DK:(g * HG + i + 1) * DK] = ot[i].T
            rinv_full[b, :, g * HG + i] = 1.0 / rs[i]

    attn_flat = attn_full.reshape(B * S, D)
    rinv_flat = rinv_full.reshape(B * S, H)
    q_flat = q.reshape(B * S, D)
    RPC = (B * S) // NCORES
    in_maps2 = []
    for c in range(NCORES):
        rows = slice(c * RPC, (c + 1) * RPC)
        in_maps2.append({
            "attn": np.ascontiguousarray(attn_flat[rows]),
            "rinv": np.ascontiguousarray(rinv_flat[rows]),
            "resid": np.ascontiguousarray(q_flat[rows]),
            "gamma": gamma,
            "beta": beta,
        })
    res2 = run_bass_kernel_spmd(
        nc_ln, in_maps2, core_ids=list(range(NCORES)), trace=trace
    )
    out = np.concatenate(
        [res2.results[c]["out"] for c in range(NCORES)], axis=0
    ).reshape(B, S, D)
    return out, res1, res2


def kernel(**inputs):
    out, _, _ = _run(inputs, trace=False)
    return out
